# revision 5
# baseline (speedup 1.0000x reference)
"""MultiHeadAttention Trainium2 kernel (8 NeuronCores, head-parallel).

Sharding: core c owns heads (2c, 2c+1) == feature slice [128c, 128c+128).
Host pre-transposes activations to x^T [E, B*S] (replicated to all cores)
and slices/transposes the weights per core. Device computes, per core:
  qT/kT [128feat, B*S]  (features on partitions -> D-on-partition for scores)
  v_aug [B*S, 65]/head  (v columns + ones column -> softmax denom for free)
  s^T[j,i] = k.q  -> exp -> attn*V via lhsT=v_aug (no transposes needed)
  second softmax over D via ones-matmul partition reduction
  partial out-projection (contraction over this core's 128 features)
Host gathers: permutes the attention scratch layout to [B,H,S,S], sums the
8 out-projection partials (the "all-reduce") and adds out_b.
"""

import os
import sys
from contextlib import ExitStack

import numpy as np

_TRN = "/opt/trn_rl_repo"
if os.path.isdir(_TRN) and _TRN not in sys.path:
    sys.path.insert(0, _TRN)

import concourse.bass as bass  # noqa: E402
import concourse.mybir as mybir  # noqa: E402
import concourse.tile as tile  # noqa: E402
from concourse import bacc  # noqa: E402
from concourse.bass_utils import run_bass_kernel_spmd  # noqa: E402

B, S, E, H = 4, 2048, 1024, 16
D = E // H          # 64
P = 128
BS = B * S          # 8192
NCORES = 8
HLOC = 2            # heads per core
IC = 256            # i-chunk width in the attention phase
NIC = S // IC       # 8
NJT = S // P        # 16 j-tiles per batch
RC = 256            # r-chunk width in the projection phase
NRC_B = S // RC     # 4 r-chunks per batch
NEO = E // P        # 8 e-tiles (contraction)
F32 = mybir.dt.float32
AF = mybir.ActivationFunctionType

_CACHE: dict = {}
LAST_RESULTS = None


def _emit(nc: bass.Bass, ctx: ExitStack, tc: tile.TileContext):
    xq = nc.dram_tensor("xq_t", [E, BS], F32, kind="ExternalInput")
    xk = nc.dram_tensor("xk_t", [E, BS], F32, kind="ExternalInput")
    xv = nc.dram_tensor("xv_t", [E, BS], F32, kind="ExternalInput")
    wq = nc.dram_tensor("wq_t", [E, P], F32, kind="ExternalInput")
    wk = nc.dram_tensor("wk_t", [E, P], F32, kind="ExternalInput")
    wv = nc.dram_tensor("wv_t", [E, P], F32, kind="ExternalInput")
    wo = nc.dram_tensor("wo_t", [P, E], F32, kind="ExternalInput")
    bq = nc.dram_tensor("bq", [P, 1], F32, kind="ExternalInput")
    bk = nc.dram_tensor("bk", [P, 1], F32, kind="ExternalInput")
    bv = nc.dram_tensor("bv", [1, P], F32, kind="ExternalInput")
    # attention scratch: [b*2+hl, ic, jp, jt, ii]
    attn_s = nc.dram_tensor("attn_s", [B * HLOC, NIC, P, NJT, IC], F32,
                            kind="ExternalOutput")
    po = nc.dram_tensor("po", [BS, E], F32, kind="ExternalOutput")

    xq_t = xq.rearrange("(eo ei) r -> ei eo r", ei=P)
    xk_t = xk.rearrange("(eo ei) r -> ei eo r", ei=P)
    xv_t = xv.rearrange("(eo ei) r -> ei eo r", ei=P)

    wpool = ctx.enter_context(tc.tile_pool(name="weights", bufs=1))
    qkv = ctx.enter_context(tc.tile_pool(name="qkv", bufs=2))
    vpool = ctx.enter_context(tc.tile_pool(name="vaug", bufs=1))
    xin = ctx.enter_context(tc.tile_pool(name="xin", bufs=2))
    expp = ctx.enter_context(tc.tile_pool(name="expp", bufs=3))
    small = ctx.enter_context(tc.tile_pool(name="small", bufs=3))
    pop = ctx.enter_context(tc.tile_pool(name="pop", bufs=2))
    ps_proj = ctx.enter_context(tc.tile_pool(name="ps_proj", bufs=2, space="PSUM"))
    ps_s = ctx.enter_context(tc.tile_pool(name="ps_s", bufs=2, space="PSUM"))
    ps_av = ctx.enter_context(tc.tile_pool(name="ps_av", bufs=2, space="PSUM"))
    ps_misc = ctx.enter_context(tc.tile_pool(name="ps_misc", bufs=2, space="PSUM"))

    # resident weights / constants
    wq_sb = wpool.tile([P, NEO, P], F32, tag="wq")
    wk_sb = wpool.tile([P, NEO, P], F32, tag="wk")
    wv_sb = wpool.tile([P, NEO, P], F32, tag="wv")
    wo_sb = wpool.tile([P, E], F32, tag="wo")
    bq_sb = wpool.tile([P, 1], F32, tag="bq")
    bk_sb = wpool.tile([P, 1], F32, tag="bk")
    bv_sb = wpool.tile([1, P], F32, tag="bv")
    ones = wpool.tile([P, P], F32, tag="ones")
    nc.sync.dma_start(out=wq_sb, in_=wq.rearrange("(eo ei) f -> ei eo f", ei=P))
    nc.sync.dma_start(out=wk_sb, in_=wk.rearrange("(eo ei) f -> ei eo f", ei=P))
    nc.sync.dma_start(out=wv_sb, in_=wv.rearrange("(eo ei) f -> ei eo f", ei=P))
    nc.sync.dma_start(out=wo_sb, in_=wo[:, :])
    nc.sync.dma_start(out=bq_sb, in_=bq[:, :])
    nc.sync.dma_start(out=bk_sb, in_=bk[:, :])
    nc.sync.dma_start(out=bv_sb, in_=bv[:, :])
    nc.vector.memset(ones, 1.0)

    # v_aug: [jp, rt(all batches), hl, 65]  (64 v columns + ones column)
    v_aug = vpool.tile([P, BS // P, HLOC, D + 1], F32, tag="vaug")
    nc.vector.memset(v_aug[:, :, :, D:D + 1], 1.0)

    for b in range(B):
        r0 = b * S
        # ---------------- projections for batch b ----------------
        qT = qkv.tile([P, S], F32, tag="qT")
        kT = qkv.tile([P, S], F32, tag="kT")
        for rc in range(NRC_B):
            rr = r0 + rc * RC
            xq_sb = xin.tile([P, NEO, RC], F32, tag="xq")
            xk_sb = xin.tile([P, NEO, RC], F32, tag="xk")
            xv_sb = xin.tile([P, NEO, RC], F32, tag="xv")
            nc.sync.dma_start(out=xq_sb, in_=xq_t[:, :, rr:rr + RC])
            nc.sync.dma_start(out=xk_sb, in_=xk_t[:, :, rr:rr + RC])
            nc.sync.dma_start(out=xv_sb, in_=xv_t[:, :, rr:rr + RC])

            pq = ps_proj.tile([P, RC], F32, tag="proj")
            for e in range(NEO):
                nc.tensor.matmul(pq, lhsT=wq_sb[:, e, :], rhs=xq_sb[:, e, :],
                                 start=(e == 0), stop=(e == NEO - 1))
            nc.scalar.activation(out=qT[:, rc * RC:(rc + 1) * RC], in_=pq,
                                 func=AF.Identity, bias=bq_sb, scale=1.0)

            pk = ps_proj.tile([P, RC], F32, tag="proj")
            for e in range(NEO):
                nc.tensor.matmul(pk, lhsT=wk_sb[:, e, :], rhs=xk_sb[:, e, :],
                                 start=(e == 0), stop=(e == NEO - 1))
            nc.scalar.activation(out=kT[:, rc * RC:(rc + 1) * RC], in_=pk,
                                 func=AF.Identity, bias=bk_sb, scale=1.0)

            for sub in range(RC // P):
                pv = ps_proj.tile([P, P], F32, tag="proj")
                for e in range(NEO):
                    nc.tensor.matmul(pv, lhsT=xv_sb[:, e, sub * P:(sub + 1) * P],
                                     rhs=wv_sb[:, e, :],
                                     start=(e == 0), stop=False)
                nc.tensor.matmul(pv, lhsT=ones[0:1, :], rhs=bv_sb,
                                 start=False, stop=True)
                rt = (r0 + rc * RC + sub * P) // P
                nc.scalar.copy(v_aug[:, rt, 0, 0:D], pv[:, 0:D])
                nc.scalar.copy(v_aug[:, rt, 1, 0:D], pv[:, D:2 * D])

        # ---------------- attention for batch b ----------------
        for icx in range(NIC):
            i0 = icx * IC
            exp_t = [expp.tile([P, NJT, IC], F32, tag="exp", name=f"exp{_hl}")
                     for _hl in range(HLOC)]
            av_ps = [ps_av.tile([D + 1, IC], F32, tag="av", name=f"av{_hl}")
                     for _hl in range(HLOC)]
            for jtp in range(NJT // 2):
                for hl in range(HLOC):
                    hsl = slice(hl * D, (hl + 1) * D)
                    pss = ps_s.tile([P, 2, IC], F32, tag="s")
                    for j2 in range(2):
                        jt = jtp * 2 + j2
                        nc.tensor.matmul(
                            pss[:, j2, :],
                            lhsT=kT[hsl, jt * P:(jt + 1) * P],
                            rhs=qT[hsl, i0:i0 + IC],
                            start=True, stop=True)
                    nc.scalar.activation(
                        out=exp_t[hl][:, jtp * 2:jtp * 2 + 2, :], in_=pss,
                        func=AF.Exp, bias=0.0, scale=1.0 / np.sqrt(D))
                    for j2 in range(2):
                        jt = jtp * 2 + j2
                        nc.tensor.matmul(
                            av_ps[hl],
                            lhsT=v_aug[:, b * NJT + jt, hl, :],
                            rhs=exp_t[hl][:, jt, :],
                            start=(jt == 0), stop=(jt == NJT - 1))

            xout = small.tile([P, IC], F32, tag="xout")
            for hl in range(HLOC):
                av_sb = small.tile([D + 1, IC], F32, tag="av_sb")
                nc.scalar.copy(av_sb, av_ps[hl])
                rec = small.tile([D + 1, IC], F32, tag="rec")
                nc.scalar.activation(out=rec[D:D + 1, :], in_=av_sb[D:D + 1, :],
                                     func=AF.Ln, bias=0.0, scale=1.0)
                nc.scalar.activation(out=rec[D:D + 1, :], in_=rec[D:D + 1, :],
                                     func=AF.Exp, bias=0.0, scale=-1.0)
                bc = ps_misc.tile([P, IC], F32, tag="misc")
                nc.tensor.matmul(bc, lhsT=ones[D:D + 1, :], rhs=rec[D:D + 1, :],
                                 start=True, stop=True)
                # normalize attention tile in place, then write out
                nc.vector.tensor_tensor(
                    out=exp_t[hl], in0=exp_t[hl],
                    in1=bc[:, None, :].to_broadcast([P, NJT, IC]),
                    op=mybir.AluOpType.mult)
                nc.sync.dma_start(out=attn_s[b * HLOC + hl, icx], in_=exp_t[hl])
                # normalized attention output for this head, then softmax on D
                oh = small.tile([D, IC], F32, tag="oh")
                nc.vector.tensor_tensor(out=oh, in0=av_sb[0:D, :], in1=bc[0:D, :],
                                        op=mybir.AluOpType.mult)
                e2 = small.tile([D, IC], F32, tag="e2")
                nc.scalar.activation(out=e2, in_=oh, func=AF.Exp,
                                     bias=0.0, scale=1.0)
                d2 = ps_misc.tile([1, IC], F32, tag="misc")
                nc.tensor.matmul(d2, lhsT=ones[0:D, 0:1], rhs=e2,
                                 start=True, stop=True)
                r2 = small.tile([1, IC], F32, tag="r2")
                nc.vector.reciprocal_approx_fast(out=r2, in_=d2)
                b2 = ps_misc.tile([D, IC], F32, tag="misc")
                nc.tensor.matmul(b2, lhsT=ones[0:1, 0:D], rhs=r2,
                                 start=True, stop=True)
                nc.vector.tensor_tensor(out=xout[hl * D:(hl + 1) * D, :],
                                        in0=e2, in1=b2,
                                        op=mybir.AluOpType.mult)

            for rs in range(IC // P):
                po_sb = pop.tile([P, E], F32, tag="po")
                for fc in range(E // 512):
                    op_ps = ps_misc.tile([P, 512], F32, tag="misc")
                    nc.tensor.matmul(op_ps,
                                     lhsT=xout[:, rs * P:(rs + 1) * P],
                                     rhs=wo_sb[:, fc * 512:(fc + 1) * 512],
                                     start=True, stop=True)
                    nc.vector.tensor_copy(po_sb[:, fc * 512:(fc + 1) * 512], op_ps)
                rr = r0 + i0 + rs * P
                nc.sync.dma_start(out=po[rr:rr + P, :], in_=po_sb)


def _build():
    if "nc" in _CACHE:
        return _CACHE["nc"]
    nc = bacc.Bacc("TRN2", target_bir_lowering=False, debug=False,
                   num_devices=NCORES)
    with ExitStack() as ctx:
        tc = ctx.enter_context(tile.TileContext(nc))
        _emit(nc, ctx, tc)
    nc.compile()
    _CACHE["nc"] = nc
    return nc


def kernel(key, query, value, wk_w, wk_b, wq_w, wq_b, wv_w, wv_b, out_w, out_b):
    global LAST_RESULTS
    key = np.ascontiguousarray(np.asarray(key, dtype=np.float32))
    query = np.ascontiguousarray(np.asarray(query, dtype=np.float32))
    value = np.ascontiguousarray(np.asarray(value, dtype=np.float32))
    wk_w = np.asarray(wk_w, dtype=np.float32)
    wq_w = np.asarray(wq_w, dtype=np.float32)
    wv_w = np.asarray(wv_w, dtype=np.float32)
    out_w = np.asarray(out_w, dtype=np.float32)
    wk_b = np.asarray(wk_b, dtype=np.float32)
    wq_b = np.asarray(wq_b, dtype=np.float32)
    wv_b = np.asarray(wv_b, dtype=np.float32)
    out_b = np.asarray(out_b, dtype=np.float32)

    nc = _build()

    xq_t = np.ascontiguousarray(query.reshape(BS, E).T)
    xk_t = np.ascontiguousarray(key.reshape(BS, E).T)
    xv_t = np.ascontiguousarray(value.reshape(BS, E).T)

    in_maps = []
    for c in range(NCORES):
        sl = slice(c * P, (c + 1) * P)
        in_maps.append({
            "xq_t": xq_t, "xk_t": xk_t, "xv_t": xv_t,
            "wq_t": np.ascontiguousarray(wq_w[sl, :].T),
            "wk_t": np.ascontiguousarray(wk_w[sl, :].T),
            "wv_t": np.ascontiguousarray(wv_w[sl, :].T),
            "wo_t": np.ascontiguousarray(out_w[:, sl].T),
            "bq": np.ascontiguousarray(wq_b[sl].reshape(P, 1)),
            "bk": np.ascontiguousarray(wk_b[sl].reshape(P, 1)),
            "bv": np.ascontiguousarray(wv_b[sl].reshape(1, P)),
        })

    res = run_bass_kernel_spmd(nc, in_maps, core_ids=list(range(NCORES)))
    LAST_RESULTS = res

    attn = np.empty((B, H, S, S), dtype=np.float32)
    for c in range(NCORES):
        arr = res.results[c]["attn_s"]
        for b in range(B):
            for hl in range(HLOC):
                blk = arr[b * HLOC + hl]          # [ic, jp, jt, ii]
                attn[b, HLOC * c + hl] = (
                    blk.transpose(0, 3, 2, 1).reshape(S, S))

    acc = np.zeros((BS, E), dtype=np.float64)
    for c in range(NCORES):
        acc += res.results[c]["po"]
    out = (acc + out_b.astype(np.float64)).astype(np.float32).reshape(B, S, E)
    return out, attn


# revision 6
# speedup vs baseline: 1.8565x; 1.8565x over previous
"""MultiHeadAttention Trainium2 kernel (8 NeuronCores, head-parallel).

Sharding: core c owns heads (2c, 2c+1) == feature slice [128c, 128c+128).
Host pre-transposes activations to x^T [E, B*S] (replicated to all cores)
and slices/transposes the weights per core. Device computes, per core:
  qT/kT [128feat, B*S]  (features on partitions -> D-on-partition for scores)
  v_aug [B*S, 65]/head  (v columns + ones column -> softmax denom for free)
  s^T[j,i] = k.q  -> exp -> attn*V via lhsT=v_aug (no transposes needed)
  second softmax over D via ones-matmul partition reduction
  partial out-projection (contraction over this core's 128 features)
All matmuls run in float32r (1-pass fp32, ~1e-4 rel err) for 2x PE rate.
Host gathers: permutes the attention scratch layout to [B,H,S,S], sums the
8 out-projection partials (the "all-reduce") and adds out_b.
"""

import os
import sys
from contextlib import ExitStack

import numpy as np

_TRN = "/opt/trn_rl_repo"
if os.path.isdir(_TRN) and _TRN not in sys.path:
    sys.path.insert(0, _TRN)

import concourse.bass as bass  # noqa: E402
import concourse.mybir as mybir  # noqa: E402
import concourse.tile as tile  # noqa: E402
from concourse import bacc  # noqa: E402
from concourse.bass_utils import run_bass_kernel_spmd  # noqa: E402

B, S, E, H = 4, 2048, 1024, 16
D = E // H          # 64
P = 128
BS = B * S          # 8192
NCORES = 8
HLOC = 2            # heads per core
IC = 256            # i-chunk width in the attention phase
NIC = S // IC       # 8
NJT = S // P        # 16 j-tiles per batch
RC = 256            # r-chunk width in the projection phase
NRC_B = S // RC     # 8 r-chunks per batch
NEO = E // P        # 8 e-tiles (contraction)
F32 = mybir.dt.float32
F32R = mybir.dt.float32r
AF = mybir.ActivationFunctionType

_CACHE: dict = {}
LAST_RESULTS = None


def _emit(nc: bass.Bass, ctx: ExitStack, tc: tile.TileContext):
    xq = nc.dram_tensor("xq_t", [E, BS], F32, kind="ExternalInput")
    xk = nc.dram_tensor("xk_t", [E, BS], F32, kind="ExternalInput")
    xv = nc.dram_tensor("xv_t", [E, BS], F32, kind="ExternalInput")
    wq = nc.dram_tensor("wq_t", [E, P], F32, kind="ExternalInput")
    wk = nc.dram_tensor("wk_t", [E, P], F32, kind="ExternalInput")
    wv = nc.dram_tensor("wv_t", [E, P], F32, kind="ExternalInput")
    wo = nc.dram_tensor("wo_t", [P, E], F32, kind="ExternalInput")
    bq = nc.dram_tensor("bq", [P, 1], F32, kind="ExternalInput")
    bk = nc.dram_tensor("bk", [P, 1], F32, kind="ExternalInput")
    bv = nc.dram_tensor("bv", [1, P], F32, kind="ExternalInput")
    ones_in = nc.dram_tensor("ones_in", [P, P], F32, kind="ExternalInput")
    vones = nc.dram_tensor("vones", [P, BS // P, HLOC, 1], F32,
                           kind="ExternalInput")
    # attention scratch: [b*2+hl, ic, jp, jt, ii]
    attn_s = nc.dram_tensor("attn_s", [B * HLOC, NIC, P, NJT, IC], F32,
                            kind="ExternalOutput")
    po = nc.dram_tensor("po", [BS, E], F32, kind="ExternalOutput")

    xq_t = xq.rearrange("(eo ei) r -> ei eo r", ei=P)
    xk_t = xk.rearrange("(eo ei) r -> ei eo r", ei=P)
    xv_t = xv.rearrange("(eo ei) r -> ei eo r", ei=P)

    wpool = ctx.enter_context(tc.tile_pool(name="weights", bufs=1))
    qkv = ctx.enter_context(tc.tile_pool(name="qkv", bufs=2))
    vpool = ctx.enter_context(tc.tile_pool(name="vaug", bufs=1))
    xin = ctx.enter_context(tc.tile_pool(name="xin", bufs=4))
    expp = ctx.enter_context(tc.tile_pool(name="expp", bufs=3))
    stg = ctx.enter_context(tc.tile_pool(name="stg", bufs=2))
    small = ctx.enter_context(tc.tile_pool(name="small", bufs=3))
    pop = ctx.enter_context(tc.tile_pool(name="pop", bufs=2))
    ps_proj = ctx.enter_context(tc.tile_pool(name="ps_proj", bufs=2, space="PSUM"))
    ps_s = ctx.enter_context(tc.tile_pool(name="ps_s", bufs=2, space="PSUM"))
    ps_av = ctx.enter_context(tc.tile_pool(name="ps_av", bufs=2, space="PSUM"))
    ps_misc = ctx.enter_context(tc.tile_pool(name="ps_misc", bufs=2, space="PSUM"))

    # resident weights / constants (f32r so matmuls run 1-pass)
    wq_sb = wpool.tile([P, NEO, P], F32R, tag="wq")
    wk_sb = wpool.tile([P, NEO, P], F32R, tag="wk")
    wv_sb = wpool.tile([P, NEO, P], F32R, tag="wv")
    wo_sb = wpool.tile([P, E], F32R, tag="wo")
    bq_sb = wpool.tile([P, 1], F32, tag="bq")
    bk_sb = wpool.tile([P, 1], F32, tag="bk")
    bv_sb = wpool.tile([1, P], F32R, tag="bv")
    ones = wpool.tile([P, P], F32R, tag="ones")
    nc.sync.dma_start(out=wq_sb,
                      in_=wq.rearrange("(eo ei) f -> ei eo f", ei=P).bitcast(F32R))
    nc.sync.dma_start(out=wk_sb,
                      in_=wk.rearrange("(eo ei) f -> ei eo f", ei=P).bitcast(F32R))
    nc.sync.dma_start(out=wv_sb,
                      in_=wv.rearrange("(eo ei) f -> ei eo f", ei=P).bitcast(F32R))
    nc.sync.dma_start(out=wo_sb, in_=wo[:, :].bitcast(F32R))
    nc.sync.dma_start(out=bq_sb, in_=bq[:, :])
    nc.sync.dma_start(out=bk_sb, in_=bk[:, :])
    nc.sync.dma_start(out=bv_sb, in_=bv[:, :].bitcast(F32R))
    nc.sync.dma_start(out=ones, in_=ones_in[:, :].bitcast(F32R))

    # v_aug: [jp, rt(all batches), hl, 65]  (64 v columns + ones column)
    v_aug = vpool.tile([P, BS // P, HLOC, D + 1], F32R, tag="vaug")
    nc.sync.dma_start(out=v_aug[:, :, :, D:D + 1],
                      in_=vones[:, :, :, :].bitcast(F32R))

    for b in range(B):
        r0 = b * S
        # ---------------- projections for batch b ----------------
        qT = qkv.tile([P, S], F32R, tag="qT")
        kT = qkv.tile([P, S], F32R, tag="kT")
        for rc in range(NRC_B):
            rr = r0 + rc * RC
            xq_sb = xin.tile([P, NEO, RC], F32R, tag="xi", name="xq_sb")
            xk_sb = xin.tile([P, NEO, RC], F32R, tag="xi", name="xk_sb")
            xv_sb = xin.tile([P, NEO, RC], F32R, tag="xi", name="xv_sb")
            nc.sync.dma_start(out=xq_sb, in_=xq_t[:, :, rr:rr + RC].bitcast(F32R))
            nc.sync.dma_start(out=xk_sb, in_=xk_t[:, :, rr:rr + RC].bitcast(F32R))
            nc.sync.dma_start(out=xv_sb, in_=xv_t[:, :, rr:rr + RC].bitcast(F32R))

            pq = ps_proj.tile([P, RC], F32, tag="proj")
            for e in range(NEO):
                nc.tensor.matmul(pq, lhsT=wq_sb[:, e, :], rhs=xq_sb[:, e, :],
                                 start=(e == 0), stop=(e == NEO - 1))
            nc.scalar.activation(out=qT[:, rc * RC:(rc + 1) * RC], in_=pq,
                                 func=AF.Identity, bias=bq_sb, scale=1.0)

            pk = ps_proj.tile([P, RC], F32, tag="proj")
            for e in range(NEO):
                nc.tensor.matmul(pk, lhsT=wk_sb[:, e, :], rhs=xk_sb[:, e, :],
                                 start=(e == 0), stop=(e == NEO - 1))
            nc.scalar.activation(out=kT[:, rc * RC:(rc + 1) * RC], in_=pk,
                                 func=AF.Identity, bias=bk_sb, scale=1.0)

            for sub in range(RC // P):
                pv = ps_proj.tile([P, P], F32, tag="proj")
                for e in range(NEO):
                    nc.tensor.matmul(pv, lhsT=xv_sb[:, e, sub * P:(sub + 1) * P],
                                     rhs=wv_sb[:, e, :],
                                     start=(e == 0), stop=False)
                nc.tensor.matmul(pv, lhsT=ones[0:1, :], rhs=bv_sb,
                                 start=False, stop=True)
                rt = (r0 + rc * RC + sub * P) // P
                nc.scalar.copy(v_aug[:, rt, 0, 0:D], pv[:, 0:D])
                nc.scalar.copy(v_aug[:, rt, 1, 0:D], pv[:, D:2 * D])

        # ---------------- attention for batch b ----------------
        for icx in range(NIC):
            i0 = icx * IC
            exp_t = [expp.tile([P, NJT, IC], F32R, tag="exp", name=f"exp{_hl}")
                     for _hl in range(HLOC)]
            av_ps = [ps_av.tile([D + 1, IC], F32, tag="av", name=f"av{_hl}")
                     for _hl in range(HLOC)]
            for jtp in range(NJT // 2):
                for hl in range(HLOC):
                    hsl = slice(hl * D, (hl + 1) * D)
                    pss = ps_s.tile([P, 2, IC], F32, tag="s")
                    for j2 in range(2):
                        jt = jtp * 2 + j2
                        nc.tensor.matmul(
                            pss[:, j2, :],
                            lhsT=kT[hsl, jt * P:(jt + 1) * P],
                            rhs=qT[hsl, i0:i0 + IC],
                            start=True, stop=True)
                    nc.scalar.activation(
                        out=exp_t[hl][:, jtp * 2:jtp * 2 + 2, :], in_=pss,
                        func=AF.Exp, bias=0.0, scale=1.0 / np.sqrt(D))
                    for j2 in range(2):
                        jt = jtp * 2 + j2
                        nc.tensor.matmul(
                            av_ps[hl],
                            lhsT=v_aug[:, b * NJT + jt, hl, :],
                            rhs=exp_t[hl][:, jt, :],
                            start=(jt == 0), stop=(jt == NJT - 1))

            xout = small.tile([P, IC], F32R, tag="xout")
            for hl in range(HLOC):
                av_sb = small.tile([D + 1, IC], F32R, tag="av_sb")
                nc.scalar.copy(av_sb, av_ps[hl])
                # denominator -> broadcast to 128 partitions -> 1/x
                bc_den = ps_misc.tile([P, IC], F32, tag="misc", name="bc_den")
                nc.tensor.matmul(bc_den, lhsT=ones[D:D + 1, :],
                                 rhs=av_sb[D:D + 1, :], start=True, stop=True)
                rec_bc = small.tile([P, IC], F32, tag="rec")
                nc.vector.reciprocal_approx_fast(out=rec_bc, in_=bc_den)
                # normalized attention tiles -> staging -> DRAM
                for half in range(2):
                    hj = NJT // 2
                    stage = stg.tile([P, hj, IC], F32, tag="stg")
                    nc.vector.tensor_tensor(
                        out=stage,
                        in0=exp_t[hl][:, half * hj:(half + 1) * hj, :].bitcast(F32),
                        in1=rec_bc[:, None, :].to_broadcast([P, hj, IC]),
                        op=mybir.AluOpType.mult)
                    nc.sync.dma_start(
                        out=attn_s[b * HLOC + hl, icx, :,
                                   half * hj:(half + 1) * hj, :],
                        in_=stage)
                # attention output for this head, then softmax over D
                oh = small.tile([D, IC], F32, tag="oh")
                nc.vector.tensor_tensor(out=oh, in0=av_sb[0:D, :].bitcast(F32),
                                        in1=rec_bc[0:D, :],
                                        op=mybir.AluOpType.mult)
                e2 = small.tile([D, IC], F32R, tag="e2")
                nc.scalar.activation(out=e2, in_=oh, func=AF.Exp,
                                     bias=0.0, scale=1.0)
                d2 = ps_misc.tile([1, IC], F32, tag="misc", name="d2")
                nc.tensor.matmul(d2, lhsT=ones[0:D, 0:1], rhs=e2,
                                 start=True, stop=True)
                d2_sb = small.tile([1, IC], F32R, tag="d2s")
                nc.scalar.copy(d2_sb, d2)
                b2_den = ps_misc.tile([D, IC], F32, tag="misc", name="b2_den")
                nc.tensor.matmul(b2_den, lhsT=ones[0:1, 0:D], rhs=d2_sb,
                                 start=True, stop=True)
                rec2 = small.tile([D, IC], F32, tag="rec2")
                nc.vector.reciprocal_approx_fast(out=rec2, in_=b2_den)
                nc.vector.tensor_tensor(out=xout[hl * D:(hl + 1) * D, :],
                                        in0=e2.bitcast(F32), in1=rec2,
                                        op=mybir.AluOpType.mult)

            for rs in range(IC // P):
                po_sb = pop.tile([P, E], F32, tag="po")
                for fc in range(E // 512):
                    op_ps = ps_misc.tile([P, 512], F32, tag="misc", name="op_ps")
                    nc.tensor.matmul(op_ps,
                                     lhsT=xout[:, rs * P:(rs + 1) * P],
                                     rhs=wo_sb[:, fc * 512:(fc + 1) * 512],
                                     start=True, stop=True)
                    nc.vector.tensor_copy(po_sb[:, fc * 512:(fc + 1) * 512], op_ps)
                rr = r0 + i0 + rs * P
                nc.sync.dma_start(out=po[rr:rr + P, :], in_=po_sb)


def _build():
    if "nc" in _CACHE:
        return _CACHE["nc"]
    nc = bacc.Bacc("TRN2", target_bir_lowering=False, debug=False,
                   num_devices=NCORES)
    with ExitStack() as ctx:
        tc = ctx.enter_context(tile.TileContext(nc))
        _emit(nc, ctx, tc)
    nc.compile()
    _CACHE["nc"] = nc
    return nc


def kernel(key, query, value, wk_w, wk_b, wq_w, wq_b, wv_w, wv_b, out_w, out_b):
    global LAST_RESULTS
    key = np.ascontiguousarray(np.asarray(key, dtype=np.float32))
    query = np.ascontiguousarray(np.asarray(query, dtype=np.float32))
    value = np.ascontiguousarray(np.asarray(value, dtype=np.float32))
    wk_w = np.asarray(wk_w, dtype=np.float32)
    wq_w = np.asarray(wq_w, dtype=np.float32)
    wv_w = np.asarray(wv_w, dtype=np.float32)
    out_w = np.asarray(out_w, dtype=np.float32)
    wk_b = np.asarray(wk_b, dtype=np.float32)
    wq_b = np.asarray(wq_b, dtype=np.float32)
    wv_b = np.asarray(wv_b, dtype=np.float32)
    out_b = np.asarray(out_b, dtype=np.float32)

    nc = _build()

    xq_t = np.ascontiguousarray(query.reshape(BS, E).T)
    xk_t = np.ascontiguousarray(key.reshape(BS, E).T)
    xv_t = np.ascontiguousarray(value.reshape(BS, E).T)
    ones_full = np.ones((P, P), dtype=np.float32)
    vones = np.ones((P, BS // P, HLOC, 1), dtype=np.float32)

    in_maps = []
    for c in range(NCORES):
        sl = slice(c * P, (c + 1) * P)
        in_maps.append({
            "xq_t": xq_t, "xk_t": xk_t, "xv_t": xv_t,
            "wq_t": np.ascontiguousarray(wq_w[sl, :].T),
            "wk_t": np.ascontiguousarray(wk_w[sl, :].T),
            "wv_t": np.ascontiguousarray(wv_w[sl, :].T),
            "wo_t": np.ascontiguousarray(out_w[:, sl].T),
            "bq": np.ascontiguousarray(wq_b[sl].reshape(P, 1)),
            "bk": np.ascontiguousarray(wk_b[sl].reshape(P, 1)),
            "bv": np.ascontiguousarray(wv_b[sl].reshape(1, P)),
            "ones_in": ones_full, "vones": vones,
        })

    res = run_bass_kernel_spmd(nc, in_maps, core_ids=list(range(NCORES)))
    LAST_RESULTS = res

    attn = np.empty((B, H, S, S), dtype=np.float32)
    for c in range(NCORES):
        arr = res.results[c]["attn_s"]
        for b in range(B):
            for hl in range(HLOC):
                blk = arr[b * HLOC + hl]          # [ic, jp, jt, ii]
                attn[b, HLOC * c + hl] = (
                    blk.transpose(0, 3, 2, 1).reshape(S, S))

    acc = np.zeros((BS, E), dtype=np.float64)
    for c in range(NCORES):
        acc += res.results[c]["po"]
    out = (acc + out_b.astype(np.float64)).astype(np.float32).reshape(B, S, E)
    return out, attn


# revision 8
# speedup vs baseline: 1.9361x; 1.0429x over previous
"""MultiHeadAttention Trainium2 kernel (8 NeuronCores, head-parallel).

Sharding: core c owns heads (2c, 2c+1) == feature slice [128c, 128c+128).
Host pre-transposes activations to x^T [E, B*S] (replicated to all cores)
and slices/transposes the weights per core. Device computes, per core:
  qT/kT [128feat, B*S]  (features on partitions -> D-on-partition for scores)
  v^T then PE-transpose -> v_aug [B*S, 65]/head (v cols + ones col -> denom)
  s^T[j,i] = k.q (f32r) -> exp (bf16) -> attn*V via lhsT=v_aug (bf16)
  second softmax over D via ones-matmul partition reduction
  partial out-projection (contraction over this core's 128 features)
Matmuls run float32r (1-pass fp32) / bf16; attention weights leave in f32.
Host gathers: permutes the attention scratch layout to [B,H,S,S], sums the
8 out-projection partials (the "all-reduce") and adds out_b.
"""

import os
import sys
from contextlib import ExitStack

import numpy as np
import ml_dtypes

_TRN = "/opt/trn_rl_repo"
if os.path.isdir(_TRN) and _TRN not in sys.path:
    sys.path.insert(0, _TRN)

import concourse.bass as bass  # noqa: E402
import concourse.mybir as mybir  # noqa: E402
import concourse.tile as tile  # noqa: E402
from concourse import bacc  # noqa: E402
from concourse.bass_utils import run_bass_kernel_spmd  # noqa: E402

B, S, E, H = 4, 2048, 1024, 16
D = E // H          # 64
P = 128
BS = B * S          # 8192
NCORES = 8
HLOC = 2            # heads per core
IC = 512            # i-chunk width in the attention phase
NIC = S // IC       # 4
NJT = S // P        # 16 j-tiles per batch
RC = 512            # r-chunk width in the projection phase
NRC_B = S // RC     # 4 r-chunks per batch
NEO = E // P        # 8 e-tiles (contraction)
EH = 4              # e-tiles per xin tile (half)
F32 = mybir.dt.float32
F32R = mybir.dt.float32r
BF16 = mybir.dt.bfloat16
AF = mybir.ActivationFunctionType

_CACHE: dict = {}
LAST_RESULTS = None


def _emit(nc: bass.Bass, ctx: ExitStack, tc: tile.TileContext):
    xq = nc.dram_tensor("xq_t", [E, BS], F32, kind="ExternalInput")
    xk = nc.dram_tensor("xk_t", [E, BS], F32, kind="ExternalInput")
    xv = nc.dram_tensor("xv_t", [E, BS], F32, kind="ExternalInput")
    wq = nc.dram_tensor("wq_t", [E, P], F32, kind="ExternalInput")
    wk = nc.dram_tensor("wk_t", [E, P], F32, kind="ExternalInput")
    wv = nc.dram_tensor("wv_t", [E, P], F32, kind="ExternalInput")
    wo = nc.dram_tensor("wo_t", [P, E], F32, kind="ExternalInput")
    bq = nc.dram_tensor("bq", [P, 1], F32, kind="ExternalInput")
    bk = nc.dram_tensor("bk", [P, 1], F32, kind="ExternalInput")
    bv = nc.dram_tensor("bv", [P, 1], F32, kind="ExternalInput")
    ones_in = nc.dram_tensor("ones_in", [P, P], F32, kind="ExternalInput")
    ident_in = nc.dram_tensor("ident_in", [P, P], BF16, kind="ExternalInput")
    vones = nc.dram_tensor("vones", [P, BS // P, HLOC, 1], BF16,
                           kind="ExternalInput")
    # attention scratch: [b*2+hl, ic, jp, jt, ii]
    attn_s = nc.dram_tensor("attn_s", [B * HLOC, NIC, P, NJT, IC], F32,
                            kind="ExternalOutput")
    po = nc.dram_tensor("po", [BS, E], F32, kind="ExternalOutput")

    xq_t = xq.rearrange("(eo ei) r -> ei eo r", ei=P)
    xk_t = xk.rearrange("(eo ei) r -> ei eo r", ei=P)
    xv_t = xv.rearrange("(eo ei) r -> ei eo r", ei=P)

    wpool = ctx.enter_context(tc.tile_pool(name="weights", bufs=1))
    qkv = ctx.enter_context(tc.tile_pool(name="qkv", bufs=2))
    vtp = ctx.enter_context(tc.tile_pool(name="vtp", bufs=1))
    vpool = ctx.enter_context(tc.tile_pool(name="vaug", bufs=1))
    xin = ctx.enter_context(tc.tile_pool(name="xin", bufs=5))
    expp = ctx.enter_context(tc.tile_pool(name="expp", bufs=3))
    stg = ctx.enter_context(tc.tile_pool(name="stg", bufs=2))
    small = ctx.enter_context(tc.tile_pool(name="small", bufs=2))
    pop = ctx.enter_context(tc.tile_pool(name="pop", bufs=2))
    ps_proj = ctx.enter_context(tc.tile_pool(name="ps_proj", bufs=2, space="PSUM"))
    ps_s = ctx.enter_context(tc.tile_pool(name="ps_s", bufs=2, space="PSUM"))
    ps_av = ctx.enter_context(tc.tile_pool(name="ps_av", bufs=2, space="PSUM"))
    ps_misc = ctx.enter_context(tc.tile_pool(name="ps_misc", bufs=2, space="PSUM"))

    # resident weights / constants (f32r so matmuls run 1-pass)
    wq_sb = wpool.tile([P, NEO, P], F32R, tag="wq")
    wk_sb = wpool.tile([P, NEO, P], F32R, tag="wk")
    wv_sb = wpool.tile([P, NEO, P], F32R, tag="wv")
    wo_sb = wpool.tile([P, E], F32R, tag="wo")
    bq_sb = wpool.tile([P, 1], F32, tag="bq")
    bk_sb = wpool.tile([P, 1], F32, tag="bk")
    bv_sb = wpool.tile([P, 1], F32, tag="bv")
    ones = wpool.tile([P, P], F32R, tag="ones")
    ident = wpool.tile([P, P], BF16, tag="ident")
    nc.sync.dma_start(out=wq_sb,
                      in_=wq.rearrange("(eo ei) f -> ei eo f", ei=P).bitcast(F32R))
    nc.sync.dma_start(out=wk_sb,
                      in_=wk.rearrange("(eo ei) f -> ei eo f", ei=P).bitcast(F32R))
    nc.sync.dma_start(out=wv_sb,
                      in_=wv.rearrange("(eo ei) f -> ei eo f", ei=P).bitcast(F32R))
    nc.sync.dma_start(out=wo_sb, in_=wo[:, :].bitcast(F32R))
    nc.sync.dma_start(out=bq_sb, in_=bq[:, :])
    nc.sync.dma_start(out=bk_sb, in_=bk[:, :])
    nc.sync.dma_start(out=bv_sb, in_=bv[:, :])
    nc.sync.dma_start(out=ones, in_=ones_in[:, :].bitcast(F32R))
    nc.sync.dma_start(out=ident, in_=ident_in[:, :])

    # v_aug: [jp, rt(all batches), hl, 65]  (64 v columns + ones column)
    v_aug = vpool.tile([P, BS // P, HLOC, D + 1], BF16, tag="vaug")
    nc.sync.dma_start(out=v_aug[:, :, :, D:D + 1], in_=vones[:, :, :, :])

    def load_x(xt, rr, nm):
        ta = xin.tile([P, EH, RC], F32R, tag="xi", name=f"{nm}a")
        tb = xin.tile([P, EH, RC], F32R, tag="xi", name=f"{nm}b")
        nc.sync.dma_start(out=ta, in_=xt[:, 0:EH, rr:rr + RC].bitcast(F32R))
        nc.sync.dma_start(out=tb, in_=xt[:, EH:NEO, rr:rr + RC].bitcast(F32R))
        return (ta, tb)

    def mm_proj(psum, w_or_x, x2, first_last=True):
        for e in range(NEO):
            t = x2[e // EH]
            nc.tensor.matmul(psum, lhsT=w_or_x[:, e, :], rhs=t[:, e % EH, :],
                             start=(e == 0), stop=(e == NEO - 1))

    for b in range(B):
        r0 = b * S
        # ---------------- projections for batch b ----------------
        qT = qkv.tile([P, S], F32R, tag="qT")
        kT = qkv.tile([P, S], F32R, tag="kT")
        vT = vtp.tile([P, S], BF16, tag="vT")
        for rc in range(NRC_B):
            rr = r0 + rc * RC
            xq2 = load_x(xq_t, rr, "xq")
            xk2 = load_x(xk_t, rr, "xk")
            xv2 = load_x(xv_t, rr, "xv")

            pq = ps_proj.tile([P, RC], F32, tag="proj", name="pq")
            mm_proj(pq, wq_sb, xq2)
            nc.scalar.activation(out=qT[:, rc * RC:(rc + 1) * RC], in_=pq,
                                 func=AF.Identity, bias=bq_sb, scale=1.0)

            pk = ps_proj.tile([P, RC], F32, tag="proj", name="pk")
            mm_proj(pk, wk_sb, xk2)
            nc.scalar.activation(out=kT[:, rc * RC:(rc + 1) * RC], in_=pk,
                                 func=AF.Identity, bias=bk_sb, scale=1.0)

            pv = ps_proj.tile([P, RC], F32, tag="proj", name="pv")
            mm_proj(pv, wv_sb, xv2)
            nc.scalar.activation(out=vT[:, rc * RC:(rc + 1) * RC], in_=pv,
                                 func=AF.Identity, bias=bv_sb, scale=1.0)
            # transpose v^T -> v_aug blocks
            for sub in range(RC // P):
                rt = (rc * RC + sub * P) // P
                tps = ps_misc.tile([P, P], BF16, tag="misc", name="tps")
                nc.tensor.transpose(tps, vT[:, rt * P:(rt + 1) * P], ident)
                rtg = b * NJT + rt
                nc.scalar.copy(v_aug[:, rtg, 0, 0:D], tps[:, 0:D])
                nc.scalar.copy(v_aug[:, rtg, 1, 0:D], tps[:, D:2 * D])

        # ---------------- attention for batch b ----------------
        for icx in range(NIC):
            i0 = icx * IC
            exp_t = [expp.tile([P, NJT, IC], BF16, tag="exp", name=f"exp{_hl}")
                     for _hl in range(HLOC)]
            av_ps = [ps_av.tile([D + 1, IC], F32, tag="av", name=f"av{_hl}")
                     for _hl in range(HLOC)]
            for jt in range(NJT):
                for hl in range(HLOC):
                    hsl = slice(hl * D, (hl + 1) * D)
                    pss = ps_s.tile([P, IC], F32, tag="s")
                    nc.tensor.matmul(
                        pss,
                        lhsT=kT[hsl, jt * P:(jt + 1) * P],
                        rhs=qT[hsl, i0:i0 + IC],
                        start=True, stop=True)
                    nc.scalar.activation(
                        out=exp_t[hl][:, jt, :], in_=pss,
                        func=AF.Exp, bias=0.0, scale=1.0 / np.sqrt(D))
                    nc.tensor.matmul(
                        av_ps[hl],
                        lhsT=v_aug[:, b * NJT + jt, hl, :],
                        rhs=exp_t[hl][:, jt, :],
                        start=(jt == 0), stop=(jt == NJT - 1))

            xout = small.tile([P, IC], F32R, tag="xout")
            for hl in range(HLOC):
                av_sb = small.tile([D + 1, IC], F32R, tag="sm_r", name="av_sb")
                nc.scalar.copy(av_sb, av_ps[hl])
                # denominator -> broadcast to 128 partitions -> 1/x
                bc_den = ps_misc.tile([P, IC], F32, tag="misc", name="bc_den")
                nc.tensor.matmul(bc_den, lhsT=ones[D:D + 1, :],
                                 rhs=av_sb[D:D + 1, :], start=True, stop=True)
                rec_bc = small.tile([P, IC], F32, tag="rec")
                nc.vector.reciprocal_approx_fast(out=rec_bc, in_=bc_den)
                # normalized attention tiles -> staging -> DRAM
                nq = 4
                for quar in range(nq):
                    hj = NJT // nq
                    stage = stg.tile([P, hj, IC], F32, tag="stg")
                    nc.vector.tensor_tensor(
                        out=stage,
                        in0=exp_t[hl][:, quar * hj:(quar + 1) * hj, :],
                        in1=rec_bc[:, None, :].to_broadcast([P, hj, IC]),
                        op=mybir.AluOpType.mult)
                    nc.sync.dma_start(
                        out=attn_s[b * HLOC + hl, icx, :,
                                   quar * hj:(quar + 1) * hj, :],
                        in_=stage)
                # attention output for this head, then softmax over D
                oh = small.tile([D, IC], F32, tag="sm_f", name="oh")
                nc.vector.tensor_tensor(out=oh, in0=av_sb[0:D, :].bitcast(F32),
                                        in1=rec_bc[0:D, :],
                                        op=mybir.AluOpType.mult)
                e2 = small.tile([D, IC], F32R, tag="e2")
                nc.scalar.activation(out=e2, in_=oh, func=AF.Exp,
                                     bias=0.0, scale=1.0)
                d2 = ps_misc.tile([1, IC], F32, tag="misc", name="d2")
                nc.tensor.matmul(d2, lhsT=ones[0:D, 0:1], rhs=e2,
                                 start=True, stop=True)
                d2_sb = small.tile([1, IC], F32R, tag="sm_r", name="d2_sb")
                nc.scalar.copy(d2_sb, d2)
                b2_den = ps_misc.tile([D, IC], F32, tag="misc", name="b2_den")
                nc.tensor.matmul(b2_den, lhsT=ones[0:1, 0:D], rhs=d2_sb,
                                 start=True, stop=True)
                rec2 = small.tile([D, IC], F32, tag="sm_f", name="rec2")
                nc.vector.reciprocal_approx_fast(out=rec2, in_=b2_den)
                nc.vector.tensor_tensor(out=xout[hl * D:(hl + 1) * D, :],
                                        in0=e2.bitcast(F32), in1=rec2,
                                        op=mybir.AluOpType.mult)

            for rs in range(IC // P):
                po_sb = pop.tile([P, E], F32, tag="po")
                for fc in range(E // 512):
                    op_ps = ps_misc.tile([P, 512], F32, tag="misc", name="op_ps")
                    nc.tensor.matmul(op_ps,
                                     lhsT=xout[:, rs * P:(rs + 1) * P],
                                     rhs=wo_sb[:, fc * 512:(fc + 1) * 512],
                                     start=True, stop=True)
                    nc.vector.tensor_copy(po_sb[:, fc * 512:(fc + 1) * 512], op_ps)
                rr = r0 + i0 + rs * P
                nc.sync.dma_start(out=po[rr:rr + P, :], in_=po_sb)


def _build():
    if "nc" in _CACHE:
        return _CACHE["nc"]
    nc = bacc.Bacc("TRN2", target_bir_lowering=False, debug=False,
                   num_devices=NCORES)
    with ExitStack() as ctx:
        tc = ctx.enter_context(tile.TileContext(nc))
        _emit(nc, ctx, tc)
    nc.compile()
    _CACHE["nc"] = nc
    return nc


def kernel(key, query, value, wk_w, wk_b, wq_w, wq_b, wv_w, wv_b, out_w, out_b):
    global LAST_RESULTS
    key = np.ascontiguousarray(np.asarray(key, dtype=np.float32))
    query = np.ascontiguousarray(np.asarray(query, dtype=np.float32))
    value = np.ascontiguousarray(np.asarray(value, dtype=np.float32))
    wk_w = np.asarray(wk_w, dtype=np.float32)
    wq_w = np.asarray(wq_w, dtype=np.float32)
    wv_w = np.asarray(wv_w, dtype=np.float32)
    out_w = np.asarray(out_w, dtype=np.float32)
    wk_b = np.asarray(wk_b, dtype=np.float32)
    wq_b = np.asarray(wq_b, dtype=np.float32)
    wv_b = np.asarray(wv_b, dtype=np.float32)
    out_b = np.asarray(out_b, dtype=np.float32)

    nc = _build()

    xq_t = np.ascontiguousarray(query.reshape(BS, E).T)
    xk_t = np.ascontiguousarray(key.reshape(BS, E).T)
    xv_t = np.ascontiguousarray(value.reshape(BS, E).T)
    ones_full = np.ones((P, P), dtype=np.float32)
    ident = np.eye(P, dtype=np.float32).astype(ml_dtypes.bfloat16)
    vones = np.ones((P, BS // P, HLOC, 1), dtype=np.float32).astype(
        ml_dtypes.bfloat16)

    in_maps = []
    for c in range(NCORES):
        sl = slice(c * P, (c + 1) * P)
        in_maps.append({
            "xq_t": xq_t, "xk_t": xk_t, "xv_t": xv_t,
            "wq_t": np.ascontiguousarray(wq_w[sl, :].T),
            "wk_t": np.ascontiguousarray(wk_w[sl, :].T),
            "wv_t": np.ascontiguousarray(wv_w[sl, :].T),
            "wo_t": np.ascontiguousarray(out_w[:, sl].T),
            "bq": np.ascontiguousarray(wq_b[sl].reshape(P, 1)),
            "bk": np.ascontiguousarray(wk_b[sl].reshape(P, 1)),
            "bv": np.ascontiguousarray(wv_b[sl].reshape(P, 1)),
            "ones_in": ones_full, "ident_in": ident, "vones": vones,
        })

    res = run_bass_kernel_spmd(nc, in_maps, core_ids=list(range(NCORES)))
    LAST_RESULTS = res

    attn = np.empty((B, H, S, S), dtype=np.float32)
    for c in range(NCORES):
        arr = res.results[c]["attn_s"]
        for b in range(B):
            for hl in range(HLOC):
                blk = arr[b * HLOC + hl]          # [ic, jp, jt, ii]
                attn[b, HLOC * c + hl] = (
                    blk.transpose(0, 3, 2, 1).reshape(S, S))

    acc = np.zeros((BS, E), dtype=np.float64)
    for c in range(NCORES):
        acc += res.results[c]["po"]
    out = (acc + out_b.astype(np.float64)).astype(np.float32).reshape(B, S, E)
    return out, attn


# revision 9
# speedup vs baseline: 1.9444x; 1.0043x over previous
"""MultiHeadAttention Trainium2 kernel (8 NeuronCores, head-parallel).

Sharding: core c owns heads (2c, 2c+1) == feature slice [128c, 128c+128).
Host pre-transposes activations to x^T [E, B*S] (replicated to all cores)
and slices/transposes the weights per core. Device computes, per core:
  qT/kT [128feat, B*S]  (features on partitions -> D-on-partition for scores)
  v^T then PE-transpose -> v_aug [B*S, 65]/head (v cols + ones col -> denom)
  s^T[j,i] = k.q (f32r) -> exp (bf16) -> attn*V via lhsT=v_aug (bf16)
  second softmax over D via ones-matmul partition reduction
  partial out-projection (contraction over this core's 128 features)
Matmuls run float32r (1-pass fp32) / bf16; attention weights leave in f32.
Host gathers: permutes the attention scratch layout to [B,H,S,S], sums the
8 out-projection partials (the "all-reduce") and adds out_b.
"""

import os
import sys
from contextlib import ExitStack

import numpy as np
import ml_dtypes

_TRN = "/opt/trn_rl_repo"
if os.path.isdir(_TRN) and _TRN not in sys.path:
    sys.path.insert(0, _TRN)

import concourse.bass as bass  # noqa: E402
import concourse.mybir as mybir  # noqa: E402
import concourse.tile as tile  # noqa: E402
from concourse import bacc  # noqa: E402
from concourse.bass_utils import run_bass_kernel_spmd  # noqa: E402

B, S, E, H = 4, 2048, 1024, 16
D = E // H          # 64
P = 128
BS = B * S          # 8192
NCORES = 8
HLOC = 2            # heads per core
IC = 512            # i-chunk width in the attention phase
NIC = S // IC       # 4
NJT = S // P        # 16 j-tiles per batch
RC = 512            # r-chunk width in the projection phase
NRC_B = S // RC     # 4 r-chunks per batch
NEO = E // P        # 8 e-tiles (contraction)
EH = 4              # e-tiles per xin tile (half)
F32 = mybir.dt.float32
F32R = mybir.dt.float32r
BF16 = mybir.dt.bfloat16
AF = mybir.ActivationFunctionType

_CACHE: dict = {}
LAST_RESULTS = None


def _emit(nc: bass.Bass, ctx: ExitStack, tc: tile.TileContext):
    xq = nc.dram_tensor("xq_t", [E, BS], F32, kind="ExternalInput")
    xk = nc.dram_tensor("xk_t", [E, BS], F32, kind="ExternalInput")
    xv = nc.dram_tensor("xv_t", [E, BS], F32, kind="ExternalInput")
    wq = nc.dram_tensor("wq_t", [E, P], F32, kind="ExternalInput")
    wk = nc.dram_tensor("wk_t", [E, P], F32, kind="ExternalInput")
    wv = nc.dram_tensor("wv_t", [E, P], F32, kind="ExternalInput")
    wo = nc.dram_tensor("wo_t", [P, E], F32, kind="ExternalInput")
    bq = nc.dram_tensor("bq", [P, 1], F32, kind="ExternalInput")
    bk = nc.dram_tensor("bk", [P, 1], F32, kind="ExternalInput")
    bv = nc.dram_tensor("bv", [P, 1], F32, kind="ExternalInput")
    ones_in = nc.dram_tensor("ones_in", [P, P], F32, kind="ExternalInput")
    ident_in = nc.dram_tensor("ident_in", [P, P], BF16, kind="ExternalInput")
    vones = nc.dram_tensor("vones", [P, BS // P, HLOC, 1], BF16,
                           kind="ExternalInput")
    # attention scratch: [b*2+hl, ic, jp, jt, ii]
    attn_s = nc.dram_tensor("attn_s", [B * HLOC, NIC, P, NJT, IC], F32,
                            kind="ExternalOutput")
    po = nc.dram_tensor("po", [BS, E], F32, kind="ExternalOutput")

    xq_t = xq.rearrange("(eo ei) r -> ei eo r", ei=P)
    xk_t = xk.rearrange("(eo ei) r -> ei eo r", ei=P)
    xv_t = xv.rearrange("(eo ei) r -> ei eo r", ei=P)

    wpool = ctx.enter_context(tc.tile_pool(name="weights", bufs=1))
    qkv = ctx.enter_context(tc.tile_pool(name="qkv", bufs=2))
    vtp = ctx.enter_context(tc.tile_pool(name="vtp", bufs=1))
    vpool = ctx.enter_context(tc.tile_pool(name="vaug", bufs=1))
    xin = ctx.enter_context(tc.tile_pool(name="xin", bufs=5))
    expp = ctx.enter_context(tc.tile_pool(name="expp", bufs=3))
    stg = ctx.enter_context(tc.tile_pool(name="stg", bufs=2))
    small = ctx.enter_context(tc.tile_pool(name="small", bufs=2))
    pop = ctx.enter_context(tc.tile_pool(name="pop", bufs=2))
    ps_proj = ctx.enter_context(tc.tile_pool(name="ps_proj", bufs=1, space="PSUM"))
    ps_s = ctx.enter_context(tc.tile_pool(name="ps_s", bufs=3, space="PSUM"))
    ps_av = ctx.enter_context(tc.tile_pool(name="ps_av", bufs=3, space="PSUM"))
    ps_misc = ctx.enter_context(tc.tile_pool(name="ps_misc", bufs=1, space="PSUM"))

    # resident weights / constants (f32r so matmuls run 1-pass)
    wq_sb = wpool.tile([P, NEO, P], F32R, tag="wq")
    wk_sb = wpool.tile([P, NEO, P], F32R, tag="wk")
    wv_sb = wpool.tile([P, NEO, P], F32R, tag="wv")
    wo_sb = wpool.tile([P, E], F32R, tag="wo")
    bq_sb = wpool.tile([P, 1], F32, tag="bq")
    bk_sb = wpool.tile([P, 1], F32, tag="bk")
    bv_sb = wpool.tile([P, 1], F32, tag="bv")
    ones = wpool.tile([P, P], F32R, tag="ones")
    ident = wpool.tile([P, P], BF16, tag="ident")
    nc.sync.dma_start(out=wq_sb,
                      in_=wq.rearrange("(eo ei) f -> ei eo f", ei=P).bitcast(F32R))
    nc.sync.dma_start(out=wk_sb,
                      in_=wk.rearrange("(eo ei) f -> ei eo f", ei=P).bitcast(F32R))
    nc.sync.dma_start(out=wv_sb,
                      in_=wv.rearrange("(eo ei) f -> ei eo f", ei=P).bitcast(F32R))
    nc.sync.dma_start(out=wo_sb, in_=wo[:, :].bitcast(F32R))
    nc.sync.dma_start(out=bq_sb, in_=bq[:, :])
    nc.sync.dma_start(out=bk_sb, in_=bk[:, :])
    nc.sync.dma_start(out=bv_sb, in_=bv[:, :])
    nc.sync.dma_start(out=ones, in_=ones_in[:, :].bitcast(F32R))
    nc.sync.dma_start(out=ident, in_=ident_in[:, :])

    # v_aug: [jp, rt(all batches), hl, 65]  (64 v columns + ones column)
    v_aug = vpool.tile([P, BS // P, HLOC, D + 1], BF16, tag="vaug")
    nc.sync.dma_start(out=v_aug[:, :, :, D:D + 1], in_=vones[:, :, :, :])

    def load_x(xt, rr, nm):
        ta = xin.tile([P, EH, RC], F32R, tag="xi", name=f"{nm}a")
        tb = xin.tile([P, EH, RC], F32R, tag="xi", name=f"{nm}b")
        nc.sync.dma_start(out=ta, in_=xt[:, 0:EH, rr:rr + RC].bitcast(F32R))
        nc.sync.dma_start(out=tb, in_=xt[:, EH:NEO, rr:rr + RC].bitcast(F32R))
        return (ta, tb)

    def mm_proj(psum, w_or_x, x2, first_last=True):
        for e in range(NEO):
            t = x2[e // EH]
            nc.tensor.matmul(psum, lhsT=w_or_x[:, e, :], rhs=t[:, e % EH, :],
                             start=(e == 0), stop=(e == NEO - 1))

    for b in range(B):
        r0 = b * S
        # ---------------- projections for batch b ----------------
        qT = qkv.tile([P, S], F32R, tag="qT")
        kT = qkv.tile([P, S], F32R, tag="kT")
        vT = vtp.tile([P, S], BF16, tag="vT")
        for rc in range(NRC_B):
            rr = r0 + rc * RC
            xq2 = load_x(xq_t, rr, "xq")
            xk2 = load_x(xk_t, rr, "xk")
            xv2 = load_x(xv_t, rr, "xv")

            pq = ps_proj.tile([P, RC], F32, tag="proj", name="pq")
            mm_proj(pq, wq_sb, xq2)
            nc.scalar.activation(out=qT[:, rc * RC:(rc + 1) * RC], in_=pq,
                                 func=AF.Identity, bias=bq_sb, scale=1.0)

            pk = ps_proj.tile([P, RC], F32, tag="proj", name="pk")
            mm_proj(pk, wk_sb, xk2)
            nc.scalar.activation(out=kT[:, rc * RC:(rc + 1) * RC], in_=pk,
                                 func=AF.Identity, bias=bk_sb, scale=1.0)

            pv = ps_proj.tile([P, RC], F32, tag="proj", name="pv")
            mm_proj(pv, wv_sb, xv2)
            nc.scalar.activation(out=vT[:, rc * RC:(rc + 1) * RC], in_=pv,
                                 func=AF.Identity, bias=bv_sb, scale=1.0)
            # transpose v^T -> v_aug blocks
            for sub in range(RC // P):
                rt = (rc * RC + sub * P) // P
                tps = ps_s.tile([P, P], BF16, tag="s", name="tps")
                nc.tensor.transpose(tps, vT[:, rt * P:(rt + 1) * P], ident)
                rtg = b * NJT + rt
                nc.scalar.copy(v_aug[:, rtg, 0, 0:D], tps[:, 0:D])
                nc.scalar.copy(v_aug[:, rtg, 1, 0:D], tps[:, D:2 * D])

        # ---------------- attention for batch b ----------------
        for icx in range(NIC):
            i0 = icx * IC
            exp_t = [expp.tile([P, NJT, IC], BF16, tag="exp", name=f"exp{_hl}")
                     for _hl in range(HLOC)]
            av_ps = [ps_av.tile([D + 1, IC], F32, tag="av", name=f"av{_hl}")
                     for _hl in range(HLOC)]
            for jt in range(NJT):
                for hl in range(HLOC):
                    hsl = slice(hl * D, (hl + 1) * D)
                    pss = ps_s.tile([P, IC], F32, tag="s")
                    nc.tensor.matmul(
                        pss,
                        lhsT=kT[hsl, jt * P:(jt + 1) * P],
                        rhs=qT[hsl, i0:i0 + IC],
                        start=True, stop=True)
                    nc.scalar.activation(
                        out=exp_t[hl][:, jt, :], in_=pss,
                        func=AF.Exp, bias=0.0, scale=1.0 / np.sqrt(D))
                    nc.tensor.matmul(
                        av_ps[hl],
                        lhsT=v_aug[:, b * NJT + jt, hl, :],
                        rhs=exp_t[hl][:, jt, :],
                        start=(jt == 0), stop=(jt == NJT - 1))

            xout = small.tile([P, IC], F32R, tag="xout")
            for hl in range(HLOC):
                av_sb = small.tile([D + 1, IC], F32R, tag="sm_r", name="av_sb")
                nc.scalar.copy(av_sb, av_ps[hl])
                # denominator -> broadcast to 128 partitions -> 1/x
                bc_den = ps_misc.tile([P, IC], F32, tag="misc", name="bc_den")
                nc.tensor.matmul(bc_den, lhsT=ones[D:D + 1, :],
                                 rhs=av_sb[D:D + 1, :], start=True, stop=True)
                rec_bc = small.tile([P, IC], F32, tag="rec")
                nc.vector.reciprocal_approx_fast(out=rec_bc, in_=bc_den)
                # normalized attention tiles -> staging -> DRAM
                nq = 4
                for quar in range(nq):
                    hj = NJT // nq
                    stage = stg.tile([P, hj, IC], F32, tag="stg")
                    eng = nc.gpsimd if quar == 0 else nc.vector
                    eng.tensor_tensor(
                        out=stage,
                        in0=exp_t[hl][:, quar * hj:(quar + 1) * hj, :],
                        in1=rec_bc[:, None, :].to_broadcast([P, hj, IC]),
                        op=mybir.AluOpType.mult)
                    nc.sync.dma_start(
                        out=attn_s[b * HLOC + hl, icx, :,
                                   quar * hj:(quar + 1) * hj, :],
                        in_=stage)
                # attention output for this head, then softmax over D
                oh = small.tile([D, IC], F32, tag="sm_f", name="oh")
                nc.vector.tensor_tensor(out=oh, in0=av_sb[0:D, :].bitcast(F32),
                                        in1=rec_bc[0:D, :],
                                        op=mybir.AluOpType.mult)
                e2 = small.tile([D, IC], F32R, tag="e2")
                nc.scalar.activation(out=e2, in_=oh, func=AF.Exp,
                                     bias=0.0, scale=1.0)
                d2 = ps_misc.tile([1, IC], F32, tag="misc", name="d2")
                nc.tensor.matmul(d2, lhsT=ones[0:D, 0:1], rhs=e2,
                                 start=True, stop=True)
                d2_sb = small.tile([1, IC], F32R, tag="sm_r", name="d2_sb")
                nc.scalar.copy(d2_sb, d2)
                b2_den = ps_misc.tile([D, IC], F32, tag="misc", name="b2_den")
                nc.tensor.matmul(b2_den, lhsT=ones[0:1, 0:D], rhs=d2_sb,
                                 start=True, stop=True)
                rec2 = small.tile([D, IC], F32, tag="sm_f", name="rec2")
                nc.vector.reciprocal_approx_fast(out=rec2, in_=b2_den)
                nc.vector.tensor_tensor(out=xout[hl * D:(hl + 1) * D, :],
                                        in0=e2.bitcast(F32), in1=rec2,
                                        op=mybir.AluOpType.mult)

            for rs in range(IC // P):
                po_sb = pop.tile([P, E], F32, tag="po")
                for fc in range(E // 512):
                    op_ps = ps_av.tile([P, 512], F32, tag="av", name="op_ps")
                    nc.tensor.matmul(op_ps,
                                     lhsT=xout[:, rs * P:(rs + 1) * P],
                                     rhs=wo_sb[:, fc * 512:(fc + 1) * 512],
                                     start=True, stop=True)
                    nc.vector.tensor_copy(po_sb[:, fc * 512:(fc + 1) * 512], op_ps)
                rr = r0 + i0 + rs * P
                nc.sync.dma_start(out=po[rr:rr + P, :], in_=po_sb)


def _build():
    if "nc" in _CACHE:
        return _CACHE["nc"]
    nc = bacc.Bacc("TRN2", target_bir_lowering=False, debug=False,
                   num_devices=NCORES)
    with ExitStack() as ctx:
        tc = ctx.enter_context(tile.TileContext(nc))
        _emit(nc, ctx, tc)
    nc.compile()
    _CACHE["nc"] = nc
    return nc


def kernel(key, query, value, wk_w, wk_b, wq_w, wq_b, wv_w, wv_b, out_w, out_b):
    global LAST_RESULTS
    key = np.ascontiguousarray(np.asarray(key, dtype=np.float32))
    query = np.ascontiguousarray(np.asarray(query, dtype=np.float32))
    value = np.ascontiguousarray(np.asarray(value, dtype=np.float32))
    wk_w = np.asarray(wk_w, dtype=np.float32)
    wq_w = np.asarray(wq_w, dtype=np.float32)
    wv_w = np.asarray(wv_w, dtype=np.float32)
    out_w = np.asarray(out_w, dtype=np.float32)
    wk_b = np.asarray(wk_b, dtype=np.float32)
    wq_b = np.asarray(wq_b, dtype=np.float32)
    wv_b = np.asarray(wv_b, dtype=np.float32)
    out_b = np.asarray(out_b, dtype=np.float32)

    nc = _build()

    xq_t = np.ascontiguousarray(query.reshape(BS, E).T)
    xk_t = np.ascontiguousarray(key.reshape(BS, E).T)
    xv_t = np.ascontiguousarray(value.reshape(BS, E).T)
    ones_full = np.ones((P, P), dtype=np.float32)
    ident = np.eye(P, dtype=np.float32).astype(ml_dtypes.bfloat16)
    vones = np.ones((P, BS // P, HLOC, 1), dtype=np.float32).astype(
        ml_dtypes.bfloat16)

    in_maps = []
    for c in range(NCORES):
        sl = slice(c * P, (c + 1) * P)
        in_maps.append({
            "xq_t": xq_t, "xk_t": xk_t, "xv_t": xv_t,
            "wq_t": np.ascontiguousarray(wq_w[sl, :].T),
            "wk_t": np.ascontiguousarray(wk_w[sl, :].T),
            "wv_t": np.ascontiguousarray(wv_w[sl, :].T),
            "wo_t": np.ascontiguousarray(out_w[:, sl].T),
            "bq": np.ascontiguousarray(wq_b[sl].reshape(P, 1)),
            "bk": np.ascontiguousarray(wk_b[sl].reshape(P, 1)),
            "bv": np.ascontiguousarray(wv_b[sl].reshape(P, 1)),
            "ones_in": ones_full, "ident_in": ident, "vones": vones,
        })

    res = run_bass_kernel_spmd(nc, in_maps, core_ids=list(range(NCORES)))
    LAST_RESULTS = res

    attn = np.empty((B, H, S, S), dtype=np.float32)
    for c in range(NCORES):
        arr = res.results[c]["attn_s"]
        for b in range(B):
            for hl in range(HLOC):
                blk = arr[b * HLOC + hl]          # [ic, jp, jt, ii]
                attn[b, HLOC * c + hl] = (
                    blk.transpose(0, 3, 2, 1).reshape(S, S))

    acc = np.zeros((BS, E), dtype=np.float64)
    for c in range(NCORES):
        acc += res.results[c]["po"]
    out = (acc + out_b.astype(np.float64)).astype(np.float32).reshape(B, S, E)
    return out, attn


# revision 10
# speedup vs baseline: 2.5949x; 1.3345x over previous
"""MultiHeadAttention Trainium2 kernel (8 NeuronCores, head-parallel).

Sharding: core c owns heads (2c, 2c+1) == feature slice [128c, 128c+128).
Host pre-transposes activations to x^T [E, B*S] (replicated to all cores)
and slices/transposes the weights per core. Device computes, per core:
  qT/kT [128feat, B*S]  (features on partitions -> D-on-partition for scores)
  v^T then PE-transpose -> v_aug [B*S, 65]/head (v cols + ones col -> denom)
  s^T[j,i] = k.q (f32r) -> exp (bf16) -> attn*V via lhsT=v_aug (bf16)
  second softmax over D via ones-matmul partition reduction
  partial out-projection (contraction over this core's 128 features)
Matmuls run float32r (1-pass fp32) / bf16; attention weights leave in f32.
Host gathers: permutes the attention scratch layout to [B,H,S,S], sums the
8 out-projection partials (the "all-reduce") and adds out_b.
"""

import os
import sys
from contextlib import ExitStack

import numpy as np
import ml_dtypes

_TRN = "/opt/trn_rl_repo"
if os.path.isdir(_TRN) and _TRN not in sys.path:
    sys.path.insert(0, _TRN)

import concourse.bass as bass  # noqa: E402
import concourse.mybir as mybir  # noqa: E402
import concourse.tile as tile  # noqa: E402
from concourse import bacc  # noqa: E402
from concourse.bass_utils import run_bass_kernel_spmd  # noqa: E402

B, S, E, H = 4, 2048, 1024, 16
D = E // H          # 64
P = 128
BS = B * S          # 8192
NCORES = 8
HLOC = 2            # heads per core
IC = 512            # i-chunk width in the attention phase
NIC = S // IC       # 4
NJT = S // P        # 16 j-tiles per batch
RC = 512            # r-chunk width in the projection phase
NRC_B = S // RC     # 4 r-chunks per batch
NEO = E // P        # 8 e-tiles (contraction)
EH = 4              # e-tiles per xin tile (half)
F32 = mybir.dt.float32
F32R = mybir.dt.float32r
BF16 = mybir.dt.bfloat16
AF = mybir.ActivationFunctionType

_CACHE: dict = {}
LAST_RESULTS = None


def _emit(nc: bass.Bass, ctx: ExitStack, tc: tile.TileContext):
    xq = nc.dram_tensor("xq_t", [E, BS], BF16, kind="ExternalInput")
    xk = nc.dram_tensor("xk_t", [E, BS], BF16, kind="ExternalInput")
    xv = nc.dram_tensor("xv_t", [E, BS], BF16, kind="ExternalInput")
    wq = nc.dram_tensor("wq_t", [E, P], BF16, kind="ExternalInput")
    wk = nc.dram_tensor("wk_t", [E, P], BF16, kind="ExternalInput")
    wv = nc.dram_tensor("wv_t", [E, P], BF16, kind="ExternalInput")
    wo = nc.dram_tensor("wo_t", [P, E], F32, kind="ExternalInput")
    bq = nc.dram_tensor("bq", [P, 1], F32, kind="ExternalInput")
    bk = nc.dram_tensor("bk", [P, 1], F32, kind="ExternalInput")
    bv = nc.dram_tensor("bv", [P, 1], F32, kind="ExternalInput")
    ones_in = nc.dram_tensor("ones_in", [P, P], F32, kind="ExternalInput")
    ident_in = nc.dram_tensor("ident_in", [P, P], BF16, kind="ExternalInput")
    # attention scratch: [b*2+hl, ic, jp, jt, ii]
    attn_s = nc.dram_tensor("attn_s", [B * HLOC, NIC, P, NJT, IC], F32,
                            kind="ExternalOutput")
    po = nc.dram_tensor("po", [BS, E], F32, kind="ExternalOutput")

    xq_t = xq.rearrange("(eo ei) r -> ei eo r", ei=P)
    xk_t = xk.rearrange("(eo ei) r -> ei eo r", ei=P)
    xv_t = xv.rearrange("(eo ei) r -> ei eo r", ei=P)

    wpool = ctx.enter_context(tc.tile_pool(name="weights", bufs=1))
    qkv = ctx.enter_context(tc.tile_pool(name="qkv", bufs=2))
    vtp = ctx.enter_context(tc.tile_pool(name="vtp", bufs=1))
    vpool = ctx.enter_context(tc.tile_pool(name="vaug", bufs=1))
    xin = ctx.enter_context(tc.tile_pool(name="xin", bufs=6))
    expp = ctx.enter_context(tc.tile_pool(name="expp", bufs=4))
    stg = ctx.enter_context(tc.tile_pool(name="stg", bufs=2))
    small = ctx.enter_context(tc.tile_pool(name="small", bufs=2))
    pop = ctx.enter_context(tc.tile_pool(name="pop", bufs=2))
    ps_proj = ctx.enter_context(tc.tile_pool(name="ps_proj", bufs=1, space="PSUM"))
    ps_s = ctx.enter_context(tc.tile_pool(name="ps_s", bufs=3, space="PSUM"))
    ps_av = ctx.enter_context(tc.tile_pool(name="ps_av", bufs=3, space="PSUM"))
    ps_misc = ctx.enter_context(tc.tile_pool(name="ps_misc", bufs=1, space="PSUM"))

    # resident weights / constants (f32r so matmuls run 1-pass)
    wq_sb = wpool.tile([P, NEO, P], BF16, tag="wq")
    wk_sb = wpool.tile([P, NEO, P], BF16, tag="wk")
    wv_sb = wpool.tile([P, NEO, P], BF16, tag="wv")
    wo_sb = wpool.tile([P, E], F32R, tag="wo")
    bq_sb = wpool.tile([P, 1], F32, tag="bq")
    bk_sb = wpool.tile([P, 1], F32, tag="bk")
    bv_sb = wpool.tile([P, 1], F32, tag="bv")
    ones = wpool.tile([P, P], F32R, tag="ones")
    ident = wpool.tile([P, P], BF16, tag="ident")
    nc.sync.dma_start(out=wq_sb,
                      in_=wq.rearrange("(eo ei) f -> ei eo f", ei=P))
    nc.sync.dma_start(out=wk_sb,
                      in_=wk.rearrange("(eo ei) f -> ei eo f", ei=P))
    nc.sync.dma_start(out=wv_sb,
                      in_=wv.rearrange("(eo ei) f -> ei eo f", ei=P))
    nc.sync.dma_start(out=wo_sb, in_=wo[:, :].bitcast(F32R))
    nc.sync.dma_start(out=bq_sb, in_=bq[:, :])
    nc.sync.dma_start(out=bk_sb, in_=bk[:, :])
    nc.sync.dma_start(out=bv_sb, in_=bv[:, :])
    nc.sync.dma_start(out=ones, in_=ones_in[:, :].bitcast(F32R))
    nc.sync.dma_start(out=ident, in_=ident_in[:, :])

    # v_aug: [jp, rt(all batches), hl, 65]  (64 v columns + ones column)
    v_aug = vpool.tile([P, BS // P, HLOC, D + 1], BF16, tag="vaug")
    nc.vector.memset(v_aug[:, :, :, D:D + 1], 1.0)

    def load_x(xt, rr, nm):
        ta = xin.tile([P, EH, RC], BF16, tag="xi", name=f"{nm}a")
        tb = xin.tile([P, EH, RC], BF16, tag="xi", name=f"{nm}b")
        nc.sync.dma_start(out=ta, in_=xt[:, 0:EH, rr:rr + RC])
        nc.sync.dma_start(out=tb, in_=xt[:, EH:NEO, rr:rr + RC])
        return (ta, tb)

    def mm_proj(psum, w_or_x, x2, first_last=True):
        for e in range(NEO):
            t = x2[e // EH]
            nc.tensor.matmul(psum, lhsT=w_or_x[:, e, :], rhs=t[:, e % EH, :],
                             start=(e == 0), stop=(e == NEO - 1))

    for b in range(B):
        r0 = b * S
        # ---------------- projections for batch b ----------------
        qT = qkv.tile([P, S], BF16, tag="qT")
        kT = qkv.tile([P, S], BF16, tag="kT")
        vT = vtp.tile([P, S], BF16, tag="vT")
        for rc in range(NRC_B):
            rr = r0 + rc * RC
            xq2 = load_x(xq_t, rr, "xq")
            xk2 = load_x(xk_t, rr, "xk")
            xv2 = load_x(xv_t, rr, "xv")

            pq = ps_proj.tile([P, RC], F32, tag="proj", name="pq")
            mm_proj(pq, wq_sb, xq2)
            nc.scalar.activation(out=qT[:, rc * RC:(rc + 1) * RC], in_=pq,
                                 func=AF.Identity, bias=bq_sb, scale=1.0)

            pk = ps_proj.tile([P, RC], F32, tag="proj", name="pk")
            mm_proj(pk, wk_sb, xk2)
            nc.scalar.activation(out=kT[:, rc * RC:(rc + 1) * RC], in_=pk,
                                 func=AF.Identity, bias=bk_sb, scale=1.0)

            pv = ps_proj.tile([P, RC], F32, tag="proj", name="pv")
            mm_proj(pv, wv_sb, xv2)
            nc.scalar.activation(out=vT[:, rc * RC:(rc + 1) * RC], in_=pv,
                                 func=AF.Identity, bias=bv_sb, scale=1.0)
            # transpose v^T -> v_aug blocks
            for sub in range(RC // P):
                rt = (rc * RC + sub * P) // P
                tps = ps_s.tile([P, P], BF16, tag="s", name="tps")
                nc.tensor.transpose(tps, vT[:, rt * P:(rt + 1) * P], ident)
                rtg = b * NJT + rt
                nc.scalar.copy(v_aug[:, rtg, 0, 0:D], tps[:, 0:D])
                nc.scalar.copy(v_aug[:, rtg, 1, 0:D], tps[:, D:2 * D])

        # ---------------- attention for batch b ----------------
        for icx in range(NIC):
            i0 = icx * IC
            exp_t = [expp.tile([P, NJT, IC], BF16, tag="exp", name=f"exp{_hl}")
                     for _hl in range(HLOC)]
            av_ps = [ps_av.tile([D + 1, IC], F32, tag="av", name=f"av{_hl}")
                     for _hl in range(HLOC)]
            for jt in range(NJT):
                for hl in range(HLOC):
                    hsl = slice(hl * D, (hl + 1) * D)
                    pss = ps_s.tile([P, IC], F32, tag="s")
                    nc.tensor.matmul(
                        pss,
                        lhsT=kT[hsl, jt * P:(jt + 1) * P],
                        rhs=qT[hsl, i0:i0 + IC],
                        start=True, stop=True)
                    nc.scalar.activation(
                        out=exp_t[hl][:, jt, :], in_=pss,
                        func=AF.Exp, bias=0.0, scale=1.0 / np.sqrt(D))
                    nc.tensor.matmul(
                        av_ps[hl],
                        lhsT=v_aug[:, b * NJT + jt, hl, :],
                        rhs=exp_t[hl][:, jt, :],
                        start=(jt == 0), stop=(jt == NJT - 1))

            xout = small.tile([P, IC], F32R, tag="xout")
            for hl in range(HLOC):
                av_sb = small.tile([D + 1, IC], F32R, tag="sm_r", name="av_sb")
                nc.scalar.copy(av_sb, av_ps[hl])
                # denominator -> broadcast to 128 partitions -> 1/x
                bc_den = ps_misc.tile([P, IC], F32, tag="misc", name="bc_den")
                nc.tensor.matmul(bc_den, lhsT=ones[D:D + 1, :],
                                 rhs=av_sb[D:D + 1, :], start=True, stop=True)
                rec_bc = small.tile([P, IC], F32, tag="rec")
                nc.vector.reciprocal_approx_fast(out=rec_bc, in_=bc_den)
                # normalized attention tiles -> staging -> DRAM
                nq = 4
                for quar in range(nq):
                    hj = NJT // nq
                    stage = stg.tile([P, hj, IC], F32, tag="stg")
                    eng = nc.gpsimd if quar == 0 else nc.vector
                    eng.tensor_tensor(
                        out=stage,
                        in0=exp_t[hl][:, quar * hj:(quar + 1) * hj, :],
                        in1=rec_bc[:, None, :].to_broadcast([P, hj, IC]),
                        op=mybir.AluOpType.mult)
                    nc.sync.dma_start(
                        out=attn_s[b * HLOC + hl, icx, :,
                                   quar * hj:(quar + 1) * hj, :],
                        in_=stage)
                # attention output for this head, then softmax over D
                oh = small.tile([D, IC], F32, tag="sm_f", name="oh")
                nc.vector.tensor_tensor(out=oh, in0=av_sb[0:D, :].bitcast(F32),
                                        in1=rec_bc[0:D, :],
                                        op=mybir.AluOpType.mult)
                e2 = small.tile([D, IC], F32R, tag="e2")
                nc.scalar.activation(out=e2, in_=oh, func=AF.Exp,
                                     bias=0.0, scale=1.0)
                d2 = ps_misc.tile([1, IC], F32, tag="misc", name="d2")
                nc.tensor.matmul(d2, lhsT=ones[0:D, 0:1], rhs=e2,
                                 start=True, stop=True)
                d2_sb = small.tile([1, IC], F32R, tag="sm_r", name="d2_sb")
                nc.scalar.copy(d2_sb, d2)
                b2_den = ps_misc.tile([D, IC], F32, tag="misc", name="b2_den")
                nc.tensor.matmul(b2_den, lhsT=ones[0:1, 0:D], rhs=d2_sb,
                                 start=True, stop=True)
                rec2 = small.tile([D, IC], F32, tag="sm_f", name="rec2")
                nc.vector.reciprocal_approx_fast(out=rec2, in_=b2_den)
                nc.vector.tensor_tensor(out=xout[hl * D:(hl + 1) * D, :],
                                        in0=e2.bitcast(F32), in1=rec2,
                                        op=mybir.AluOpType.mult)

            for rs in range(IC // P):
                po_sb = pop.tile([P, E], F32, tag="po")
                for fc in range(E // 512):
                    op_ps = ps_av.tile([P, 512], F32, tag="av", name="op_ps")
                    nc.tensor.matmul(op_ps,
                                     lhsT=xout[:, rs * P:(rs + 1) * P],
                                     rhs=wo_sb[:, fc * 512:(fc + 1) * 512],
                                     start=True, stop=True)
                    nc.vector.tensor_copy(po_sb[:, fc * 512:(fc + 1) * 512], op_ps)
                rr = r0 + i0 + rs * P
                nc.sync.dma_start(out=po[rr:rr + P, :], in_=po_sb)


def _build():
    if "nc" in _CACHE:
        return _CACHE["nc"]
    nc = bacc.Bacc("TRN2", target_bir_lowering=False, debug=False,
                   num_devices=NCORES)
    with ExitStack() as ctx:
        tc = ctx.enter_context(tile.TileContext(nc))
        _emit(nc, ctx, tc)
    nc.compile()
    _CACHE["nc"] = nc
    return nc


def kernel(key, query, value, wk_w, wk_b, wq_w, wq_b, wv_w, wv_b, out_w, out_b):
    global LAST_RESULTS
    key = np.ascontiguousarray(np.asarray(key, dtype=np.float32))
    query = np.ascontiguousarray(np.asarray(query, dtype=np.float32))
    value = np.ascontiguousarray(np.asarray(value, dtype=np.float32))
    wk_w = np.asarray(wk_w, dtype=np.float32)
    wq_w = np.asarray(wq_w, dtype=np.float32)
    wv_w = np.asarray(wv_w, dtype=np.float32)
    out_w = np.asarray(out_w, dtype=np.float32)
    wk_b = np.asarray(wk_b, dtype=np.float32)
    wq_b = np.asarray(wq_b, dtype=np.float32)
    wv_b = np.asarray(wv_b, dtype=np.float32)
    out_b = np.asarray(out_b, dtype=np.float32)

    nc = _build()

    bf = ml_dtypes.bfloat16
    xq_t = np.ascontiguousarray(query.reshape(BS, E).T.astype(bf))
    xk_t = np.ascontiguousarray(key.reshape(BS, E).T.astype(bf))
    xv_t = np.ascontiguousarray(value.reshape(BS, E).T.astype(bf))
    ones_full = np.ones((P, P), dtype=np.float32)
    ident = np.eye(P, dtype=np.float32).astype(bf)

    in_maps = []
    for c in range(NCORES):
        sl = slice(c * P, (c + 1) * P)
        in_maps.append({
            "xq_t": xq_t, "xk_t": xk_t, "xv_t": xv_t,
            "wq_t": np.ascontiguousarray(wq_w[sl, :].T.astype(bf)),
            "wk_t": np.ascontiguousarray(wk_w[sl, :].T.astype(bf)),
            "wv_t": np.ascontiguousarray(wv_w[sl, :].T.astype(bf)),
            "wo_t": np.ascontiguousarray(out_w[:, sl].T),
            "bq": np.ascontiguousarray(wq_b[sl].reshape(P, 1)),
            "bk": np.ascontiguousarray(wk_b[sl].reshape(P, 1)),
            "bv": np.ascontiguousarray(wv_b[sl].reshape(P, 1)),
            "ones_in": ones_full, "ident_in": ident,
        })

    res = run_bass_kernel_spmd(nc, in_maps, core_ids=list(range(NCORES)))
    LAST_RESULTS = res

    attn = np.empty((B, H, S, S), dtype=np.float32)
    for c in range(NCORES):
        arr = res.results[c]["attn_s"]
        for b in range(B):
            for hl in range(HLOC):
                blk = arr[b * HLOC + hl]          # [ic, jp, jt, ii]
                attn[b, HLOC * c + hl] = (
                    blk.transpose(0, 3, 2, 1).reshape(S, S))

    acc = np.zeros((BS, E), dtype=np.float64)
    for c in range(NCORES):
        acc += res.results[c]["po"]
    out = (acc + out_b.astype(np.float64)).astype(np.float32).reshape(B, S, E)
    return out, attn
